# revision 25
# baseline (speedup 1.0000x reference)
"""Self-attention (CrossAttention with context=x) Bass kernel for Trainium2, 8 NeuronCores.

Problem: x:(4,2048,1024) fp32, 16 heads x 64 dim, Wq/Wk/Wv:(1024,1024), Wout:(1024,1024), bout:(1024,)
  q = x@WqT, k = x@WkT, v = x@WvT  (per head d=64, scale d**-0.25 on q and k)
  weight = softmax(q k^T), a = weight @ v, out = a@WoutT + bout

Sharding: core j handles batch j//2, head-group j%2 (8 of 16 heads).
Each core computes a partial output (its heads' contribution to out[batch]);
the host sums core pairs and adds bout.

V3 design (per core, bf16 compute, fp32 accumulate):
- Head-PAIR processing: scores row-tiled on the PE array (head A rows 0-63,
  head B rows 64-127, concurrent K=64 matmuls); values col-tiled (A cols 0-63,
  B cols 64-127, concurrent M=64 matmuls).
- exp on ScalarE from PSUM in [128,1024] tiles, double-buffered per head so
  ScalarE stays saturated. No max-subtraction (logits are small by construction).
- Softmax denominators: Zpart[p,i] += PT chunks on VectorE (bf16), column-summed
  by a ones-vector matmul; reciprocal computed in a 32x32-transposed layout so it
  runs wide across partitions; broadcast to rows via GpSimd partition_broadcast;
  normalization folded into a single PSUM->SBUF multiply per pair.
- Output projection from normalized a^T tiles; host adds pair partials + bout.
"""
import os
import sys
import types
import numpy as np

import concourse.bass as bass
import concourse.bacc as bacc
import concourse.tile as tile
from concourse import mybir
from concourse import bass_utils

BF16 = mybir.dt.bfloat16
FP32 = mybir.dt.float32
NP_BF16 = mybir.dt.np(BF16)

N_CORES = 8
LAST_EXEC_TIME_NS = None


def _install_ntff_shim():
    """Shim for missing antenv.axon_hooks so trace=True can capture NTFF profiles."""
    if "antenv.axon_hooks" in sys.modules:
        return
    try:
        import antenv  # noqa: F401
        from trn_agent_boot.trn_boot import _ntff_profile_via_ctypes
    except Exception:
        return
    hook = _ntff_profile_via_ctypes("/opt/axon/libaxon_pjrt.so")
    mod = types.ModuleType("antenv.axon_hooks")
    mod._hook = hook
    mod.set_axon_ntff_profile_hook = lambda h: setattr(mod, "_hook", h)
    mod.get_axon_ntff_profile_hook = lambda: mod._hook
    sys.modules["antenv.axon_hooks"] = mod
    sys.modules["antenv"].axon_hooks = mod


def build_nc(n=2048, c=1024, hd_l=512, num_devices=N_CORES, debug=False):
    """Build the per-core SPMD Bass graph.

    n: sequence length, c: model dim, hd_l: local head-dims (heads_l*64).
    """
    d = 64
    heads_l = hd_l // d
    pairs = heads_l // 2
    kc_x = c // 128      # contraction chunks over c
    mt_q = hd_l // 128   # qT/kT partition tiles (one per head pair)
    n_ich = n // 512     # i chunks of 512
    n_jt = n // 128      # j tiles of 128
    kc_w = hd_l // 128   # out-proj contraction chunks
    n_cch = c // 512     # out-proj N chunks
    n_ih = n // 1024     # i halves

    nc = bacc.Bacc("TRN2", target_bir_lowering=False, debug=False,
                   num_devices=num_devices)
    xT = nc.dram_tensor("xT", [c, n], BF16, kind="ExternalInput").ap()
    wqT = nc.dram_tensor("wqT", [c, hd_l], BF16, kind="ExternalInput").ap()
    wkT = nc.dram_tensor("wkT", [c, hd_l], BF16, kind="ExternalInput").ap()
    wvT = nc.dram_tensor("wvT", [c, hd_l], BF16, kind="ExternalInput").ap()
    woutT = nc.dram_tensor("woutT", [hd_l, c], BF16, kind="ExternalInput").ap()
    out = nc.dram_tensor("out", [n, c], FP32, kind="ExternalOutput").ap()
    if debug:
        dbg_at = nc.dram_tensor("dbg_at", [128, n], FP32, kind="ExternalOutput").ap()
        dbg_zp = nc.dram_tensor("dbg_zp", [128, n], FP32, kind="ExternalOutput").ap()
        dbg_zd = nc.dram_tensor("dbg_zd", [64, n], FP32, kind="ExternalOutput").ap()
        dbg_zback = nc.dram_tensor("dbg_zback", [64, n], FP32, kind="ExternalOutput").ap()
        dbg_zb = nc.dram_tensor("dbg_zb", [128, n], FP32, kind="ExternalOutput").ap()

    xT_r = xT.rearrange("(kc p) n -> kc p n", p=128)
    wqT_r = wqT.rearrange("(kc p) h -> kc p h", p=128)
    wkT_r = wkT.rearrange("(kc p) h -> kc p h", p=128)
    wvT_r = wvT.rearrange("(kc p) h -> kc p h", p=128)
    woutT_r = woutT.rearrange("(kc p) c2 -> kc p c2", p=128)

    with tile.TileContext(nc) as tc:
        persist = tc.alloc_tile_pool(name="persist", bufs=1)
        qT_sb = persist.tile([128, mt_q, n], BF16, name="qT_sb")
        kT_sb = persist.tile([128, mt_q, n], BF16, name="kT_sb")
        v_sb = persist.tile([128, n_jt, hd_l], BF16, name="v_sb")
        woutT_sb = persist.tile([128, kc_w, c], BF16, name="woutT_sb")
        atsc = [persist.tile([128, n], BF16, name=f"atsc{i}", tag=f"atsc{i}")
                for i in range(mt_q)]
        ones_sb = persist.tile([128, 1], BF16, name="ones_sb")
        nc.vector.memset(ones_sb[:, :], 1.0)

        ph1 = tc.alloc_tile_pool(name="ph1", bufs=1)
        xT_sb = ph1.tile([128, kc_x, n], BF16, name="xT_sb")
        wqT_sb = ph1.tile([128, kc_x, hd_l], BF16, name="wqT_sb")
        wkT_sb = ph1.tile([128, kc_x, hd_l], BF16, name="wkT_sb")
        wvT_sb = ph1.tile([128, kc_x, hd_l], BF16, name="wvT_sb")

        nh = n // 2
        for kc in range(kc_x):
            nc.sync.dma_start(out=wqT_sb[:, kc, :], in_=wqT_r[kc])
            nc.gpsimd.dma_start(out=wkT_sb[:, kc, :], in_=wkT_r[kc])
            eng = nc.sync if kc % 2 == 0 else nc.gpsimd
            eng.dma_start(out=xT_sb[:, kc, 0:nh], in_=xT_r[kc][:, 0:nh])
        for kc in range(kc_x):
            nc.sync.dma_start(out=xT_sb[:, kc, nh:n], in_=xT_r[kc][:, nh:n])
            nc.gpsimd.dma_start(out=wvT_sb[:, kc, :], in_=wvT_r[kc])
        for kc in range(kc_w):
            nc.gpsimd.dma_start(out=woutT_sb[:, kc, :], in_=woutT_r[kc])

        # ---- phases 1+2 interleaved: qkv chains feed the attention pipeline ----
        # Attention is ScalarE-bound (exp). qkv matmul chains are emitted inside
        # the attention loop so the PE computes projections while ScalarE exps.
        stp = tc.alloc_tile_pool(name="stp", bufs=2, space="PSUM")
        atp = tc.alloc_tile_pool(name="atp", bufs=1, space="PSUM")
        qkvp = tc.alloc_tile_pool(name="qkvp", bufs=2, space="PSUM")
        ptp = tc.alloc_tile_pool(name="ptp", bufs=16)
        zpp = tc.alloc_tile_pool(name="zpp", bufs=1)
        zdp = tc.alloc_tile_pool(name="zdp", bufs=2)
        zbp = tc.alloc_tile_pool(name="zbp", bufs=2)
        zdramp = tc.alloc_tile_pool(name="zdramp", bufs=2, space="DRAM")
        osbp = tc.alloc_tile_pool(name="osbp", bufs=3)

        def qk_chain(wsb, dst, mt, ip):
            """One q/k projection chain: 2 psum tiles, LDW amortized x2."""
            ps = [qkvp.tile([128, 512], FP32, name="ps", tag="qkv")
                  for _ in range(2)]
            for kc in range(kc_x):
                for i2 in range(2):
                    ich = ip * 2 + i2
                    nc.tensor.matmul(
                        ps[i2][:, :],
                        lhsT=wsb[:, kc, mt * 128:(mt + 1) * 128],
                        rhs=xT_sb[:, kc, ich * 512:(ich + 1) * 512],
                        start=(kc == 0), stop=(kc == kc_x - 1))
            for i2 in range(2):
                ich = ip * 2 + i2
                nc.vector.tensor_copy(
                    dst[:, mt, ich * 512:(ich + 1) * 512], ps[i2][:, :])

        def qk_chain1(wsb, dst, mt, ich):
            """Single-slot projection chain for startup priming."""
            ps1 = qkvp.tile([128, 512], FP32, name="ps1", tag="qkv")
            for kc in range(kc_x):
                nc.tensor.matmul(
                    ps1[:, :],
                    lhsT=wsb[:, kc, mt * 128:(mt + 1) * 128],
                    rhs=xT_sb[:, kc, ich * 512:(ich + 1) * 512],
                    start=(kc == 0), stop=(kc == kc_x - 1))
            nc.vector.tensor_copy(
                dst[:, mt, ich * 512:(ich + 1) * 512], ps1[:, :])

        def v_chain(nt):
            v_ps = qkvp.tile([128, hd_l], FP32, name="v_ps", tag="qkv")
            for kc in range(kc_x):
                nc.tensor.matmul(
                    v_ps[:, :],
                    lhsT=xT_sb[:, kc, nt * 128:(nt + 1) * 128],
                    rhs=wvT_sb[:, kc, :],
                    start=(kc == 0), stop=(kc == kc_x - 1))
            nc.vector.tensor_copy(v_sb[:, nt, :], v_ps[:, :])

        def op_chain(it, cch, pool=None, tag="qkv"):
            """One output-projection chain: accumulate over kc, copy out, DMA."""
            ops = (pool or qkvp).tile([128, 512], FP32, name="ops", tag=tag)
            for kc in range(kc_w):
                nc.tensor.matmul(
                    ops[:, :],
                    lhsT=atsc[kc][:, it * 128:(it + 1) * 128],
                    rhs=woutT_sb[:, kc, cch * 512:(cch + 1) * 512],
                    start=(kc == 0), stop=(kc == kc_w - 1))
            o_sb = osbp.tile([128, 512], FP32, name="o_sb", tag="osb")
            nc.vector.tensor_copy(o_sb[:, :], ops[:, :])
            nc.sync.dma_start(
                out=out[it * 128:(it + 1) * 128, cch * 512:(cch + 1) * 512],
                in_=o_sb[:, :])

        # work list of deferred qkv chains, emitted inside the attention loop
        work = []
        for ip in range(1, n_ich // 2):
            work.append(("k", 0, ip))
        for nt in range(2, n_jt):
            work.append(("v", nt))
        for ip in range(1, n_ich // 2):
            work.append(("q", 0, ip))
        for mt in range(1, mt_q):
            for ip in range(n_ich // 2):
                work.append(("q", mt, ip))
                work.append(("k", mt, ip))
        wi = 0

        def emit_work(k_items):
            nonlocal wi
            for _ in range(k_items):
                if wi >= len(work):
                    return
                item = work[wi]
                wi += 1
                if item[0] == "v":
                    v_chain(item[1])
                else:
                    wsb, dst = ((wqT_sb, qT_sb) if item[0] == "q"
                                else (wkT_sb, kT_sb))
                    qk_chain(wsb, dst, item[1], item[2])

        # prime: only what the first scores need, q and k concurrent via
        # single-slot chains (a 2-slot chain would serialize q behind k)
        qk_chain1(wqT_sb, qT_sb, 0, 0)
        qk_chain1(wkT_sb, kT_sb, 0, 0)
        qk_chain1(wqT_sb, qT_sb, 0, 1)
        qk_chain1(wkT_sb, kT_sb, 0, 1)
        v_chain(0)
        v_chain(1)

        zdance = zdp.tile([64, 1024], FP32, name="zdance", tag="zd")
        zdance_r = zdp.tile([64, 1024], FP32, name="zdance_r", tag="zdr")
        zback = zdp.tile([64, 1024], FP32, name="zback", tag="zbk")
        nc.vector.memset(zdance[:, :], 1.0)
        nc.vector.memset(zdance_r[:, :], 1.0)

        for pt in range(pairs):
            roA, roB = 0, 64
            hA, hB = 2 * pt, 2 * pt + 1
            zpA = zpp.tile([128, n], BF16, name="zpA", tag="zpA")
            zpB = zpp.tile([128, n], BF16, name="zpB", tag="zpB")
            for ih in range(n_ih):
                at = atp.tile([128, 1024], FP32, name="at", tag="at")
                pending = None  # values lag scores by one jt to keep PE fed
                for jt in range(n_jt):
                    stA = stp.tile([128, 1024], FP32, name="stA", tag="st")
                    stB = stp.tile([128, 1024], FP32, name="stB", tag="st")
                    for s2 in range(2):
                        ich = ih * 2 + s2
                        nc.tensor.matmul(
                            stA[:, s2 * 512:(s2 + 1) * 512],
                            lhsT=kT_sb[roA:roA + 64, pt, jt * 128:(jt + 1) * 128],
                            rhs=qT_sb[roA:roA + 64, pt, ich * 512:(ich + 1) * 512],
                            start=True, stop=True)
                        nc.tensor.matmul(
                            stB[:, s2 * 512:(s2 + 1) * 512],
                            lhsT=kT_sb[roB:roB + 64, pt, jt * 128:(jt + 1) * 128],
                            rhs=qT_sb[roB:roB + 64, pt, ich * 512:(ich + 1) * 512],
                            start=True, stop=True)
                    ptA = ptp.tile([128, 1024], BF16, name="ptA", tag="pt")
                    ptB = ptp.tile([128, 1024], BF16, name="ptB", tag="pt")
                    nc.scalar.activation(ptA[:, :], stA[:, :],
                                         mybir.ActivationFunctionType.Exp)
                    nc.scalar.activation(ptB[:, :], stB[:, :],
                                         mybir.ActivationFunctionType.Exp)
                    sl = slice(ih * 1024, (ih + 1) * 1024)
                    if jt == 0:
                        nc.vector.tensor_copy(zpA[:, sl], ptA[:, :])
                        nc.vector.tensor_copy(zpB[:, sl], ptB[:, :])
                    else:
                        nc.vector.tensor_add(zpA[:, sl], zpA[:, sl], ptA[:, :])
                        nc.vector.tensor_add(zpB[:, sl], zpB[:, sl], ptB[:, :])
                    def emit_values(vjt, vptA, vptB):
                        vA = v_sb[:, vjt, hA * 64:hA * 64 + 64]
                        vB = v_sb[:, vjt, hB * 64:hB * 64 + 64]
                        for s2 in range(2):
                            csl = slice(s2 * 512, (s2 + 1) * 512)
                            nc.tensor.matmul(
                                at[roA:roA + 64, csl], lhsT=vA,
                                rhs=vptA[:, csl],
                                start=(vjt == 0), stop=(vjt == n_jt - 1),
                                skip_group_check=True)
                            nc.tensor.matmul(
                                at[roB:roB + 64, csl], lhsT=vB,
                                rhs=vptB[:, csl],
                                start=(vjt == 0), stop=(vjt == n_jt - 1),
                                skip_group_check=True)
                    if pending is not None:
                        emit_values(*pending)
                    pending = (jt, ptA, ptB)
                    if wi < len(work) and (pt, ih) == (0, 0) \
                            and (work[wi][0] == "v" or work[wi][1] == 0):
                        emit_work(1)
                    elif jt % 4 == 3:
                        emit_work(1)
                    if (n_ih >= 2 and (pt, ih) == (pairs - 1, n_ih - 1)
                            and wi >= len(work)):
                        op_chain(jt // 2, jt % 2)
                if pending is not None:
                    emit_values(*pending)
                # Z for this i-half: column sums + wide reciprocal + broadcast
                for hh, zp in ((0, zpA), (1, zpB)):
                    for s2 in range(2):
                        ich = ih * 2 + s2
                        cs = qkvp.tile([1, 512], FP32, name="cs", tag="qkv")
                        nc.tensor.matmul(
                            cs[:, :],
                            lhsT=ones_sb[:, :],
                            rhs=zp[:, ich * 512:(ich + 1) * 512],
                            start=True, stop=True)
                        nc.vector.tensor_copy(
                            zdance[32 * hh:32 * hh + 1,
                                   s2 * 512:(s2 + 1) * 512],
                            cs[:, :])
                emit_work(2)
                nc.vector.transpose(zdance_r[:, :], zdance[:, :])
                rview = zdance_r.rearrange("p (b r) -> p b r", r=32)
                nc.vector.reciprocal(rview[:, :, 0:1], rview[:, :, 0:1])
                nc.vector.transpose(zback[:, :], zdance_r[:, :])
                zb = zbp.tile([128, 1024], FP32, name="zb", tag="zb")
                zd2 = zdramp.tile([2, 1024], FP32, name="zd2", tag="zd2")
                nc.sync.dma_start(out=zd2[:, :], in_=zback[0:64:32, :])
                for g in range(2):
                    row = zd2[g:g + 1, :]
                    bc = bass.AP(tensor=row.tensor, offset=row.offset,
                                 ap=[[0, 64], row.ap[1]])
                    nc.sync.dma_start(out=zb[g * 64:(g + 1) * 64, :], in_=bc)
                nc.vector.tensor_mul(atsc[pt][:, ih * 1024:(ih + 1) * 1024],
                                     at[:, :], zb[:, :])
        emit_work(len(work))
        done_its = set(range(8)) if n_ih >= 2 else set()
        alt = 0
        for it in range(n // 128):
            if it in done_its:
                continue
            for cch in range(n_cch):
                if alt % 2 == 0:
                    op_chain(it, cch)
                else:
                    op_chain(it, cch, pool=stp, tag="st")
                alt += 1
        osbp.release()
        zdramp.release()
        zbp.release()
        zdp.release()
        zpp.release()
        ptp.release()
        qkvp.release()
        atp.release()
        stp.release()
        ph1.release()
        persist.release()

    nc.compile()
    return nc


def make_in_maps(x, Wq, Wk, Wv, Wout, n=2048, c=1024, heads=16, d=64):
    """Shard + pre-transpose + cast inputs for the 8 cores."""
    s = float(d) ** -0.25
    hd_l = (heads // 2) * d
    wT = {}
    for g in range(2):
        sl = slice(g * hd_l, (g + 1) * hd_l)
        wT[g] = (
            np.ascontiguousarray((Wq[sl] * s).T).astype(NP_BF16),
            np.ascontiguousarray((Wk[sl] * s).T).astype(NP_BF16),
            np.ascontiguousarray(Wv[sl].T).astype(NP_BF16),
            np.ascontiguousarray(Wout.T[sl]).astype(NP_BF16),
        )
    in_maps = []
    for core in range(N_CORES):
        b = core // 2
        g = core % 2
        wq, wk, wv, wo = wT[g]
        in_maps.append({
            "xT": np.ascontiguousarray(x[b].T).astype(NP_BF16),
            "wqT": wq, "wkT": wk, "wvT": wv, "woutT": wo,
        })
    return in_maps


_NC_CACHE = {}


def kernel(x, Wq, Wk, Wv, Wout, bout):
    global LAST_EXEC_TIME_NS
    b, n, c = x.shape
    heads = 16
    d = 64
    hd_l = (heads // 2) * d

    if "nc" not in _NC_CACHE:
        _NC_CACHE["nc"] = build_nc(n=n, c=c, hd_l=hd_l)
    nc = _NC_CACHE["nc"]

    in_maps = make_in_maps(np.asarray(x, np.float32), np.asarray(Wq, np.float32),
                           np.asarray(Wk, np.float32), np.asarray(Wv, np.float32),
                           np.asarray(Wout, np.float32), n=n, c=c, heads=heads, d=d)

    profile = os.environ.get("BASS_KERNEL_PROFILE", "0") == "1"
    if profile:
        _install_ntff_shim()
    res = bass_utils.run_bass_kernel_spmd(
        nc, in_maps, core_ids=list(range(N_CORES)), trace=profile)
    LAST_EXEC_TIME_NS = res.exec_time_ns

    bout = np.asarray(bout, np.float32)
    out = np.empty((b, n, c), np.float32)
    for bb in range(b):
        out[bb] = res.results[2 * bb]["out"] + res.results[2 * bb + 1]["out"] + bout
    return out


# revision 26
# speedup vs baseline: 1.0087x; 1.0087x over previous
"""Self-attention (CrossAttention with context=x) Bass kernel for Trainium2, 8 NeuronCores.

Problem: x:(4,2048,1024) fp32, 16 heads x 64 dim, Wq/Wk/Wv:(1024,1024), Wout:(1024,1024), bout:(1024,)
  q = x@WqT, k = x@WkT, v = x@WvT  (per head d=64, scale d**-0.25 on q and k)
  weight = softmax(q k^T), a = weight @ v, out = a@WoutT + bout

Sharding: core j handles batch j//2, head-group j%2 (8 of 16 heads).
Each core computes a partial output (its heads' contribution to out[batch]);
the host sums core pairs and adds bout.

V3 design (per core, bf16 compute, fp32 accumulate):
- Head-PAIR processing: scores row-tiled on the PE array (head A rows 0-63,
  head B rows 64-127, concurrent K=64 matmuls); values col-tiled (A cols 0-63,
  B cols 64-127, concurrent M=64 matmuls).
- exp on ScalarE from PSUM in [128,1024] tiles, double-buffered per head so
  ScalarE stays saturated. No max-subtraction (logits are small by construction).
- Softmax denominators: Zpart[p,i] += PT chunks on VectorE (bf16), column-summed
  by a ones-vector matmul; reciprocal computed in a 32x32-transposed layout so it
  runs wide across partitions; broadcast to rows via GpSimd partition_broadcast;
  normalization folded into a single PSUM->SBUF multiply per pair.
- Output projection from normalized a^T tiles; host adds pair partials + bout.
"""
import os
import sys
import types
import numpy as np

import concourse.bass as bass
import concourse.bacc as bacc
import concourse.tile as tile
from concourse import mybir
from concourse import bass_utils

BF16 = mybir.dt.bfloat16
FP32 = mybir.dt.float32
NP_BF16 = mybir.dt.np(BF16)

N_CORES = 8
LAST_EXEC_TIME_NS = None


def _install_ntff_shim():
    """Shim for missing antenv.axon_hooks so trace=True can capture NTFF profiles."""
    if "antenv.axon_hooks" in sys.modules:
        return
    try:
        import antenv  # noqa: F401
        from trn_agent_boot.trn_boot import _ntff_profile_via_ctypes
    except Exception:
        return
    hook = _ntff_profile_via_ctypes("/opt/axon/libaxon_pjrt.so")
    mod = types.ModuleType("antenv.axon_hooks")
    mod._hook = hook
    mod.set_axon_ntff_profile_hook = lambda h: setattr(mod, "_hook", h)
    mod.get_axon_ntff_profile_hook = lambda: mod._hook
    sys.modules["antenv.axon_hooks"] = mod
    sys.modules["antenv"].axon_hooks = mod


def build_nc(n=2048, c=1024, hd_l=512, num_devices=N_CORES, debug=False):
    """Build the per-core SPMD Bass graph.

    n: sequence length, c: model dim, hd_l: local head-dims (heads_l*64).
    """
    d = 64
    heads_l = hd_l // d
    pairs = heads_l // 2
    kc_x = c // 128      # contraction chunks over c
    mt_q = hd_l // 128   # qT/kT partition tiles (one per head pair)
    n_ich = n // 512     # i chunks of 512
    n_jt = n // 128      # j tiles of 128
    kc_w = hd_l // 128   # out-proj contraction chunks
    n_cch = c // 512     # out-proj N chunks
    n_ih = n // 1024     # i halves

    nc = bacc.Bacc("TRN2", target_bir_lowering=False, debug=False,
                   num_devices=num_devices)
    xT = nc.dram_tensor("xT", [c, n], BF16, kind="ExternalInput").ap()
    wqT = nc.dram_tensor("wqT", [c, hd_l], BF16, kind="ExternalInput").ap()
    wkT = nc.dram_tensor("wkT", [c, hd_l], BF16, kind="ExternalInput").ap()
    wvT = nc.dram_tensor("wvT", [c, hd_l], BF16, kind="ExternalInput").ap()
    woutT = nc.dram_tensor("woutT", [hd_l, c], BF16, kind="ExternalInput").ap()
    out = nc.dram_tensor("out", [n, c], FP32, kind="ExternalOutput").ap()
    if debug:
        dbg_at = nc.dram_tensor("dbg_at", [128, n], FP32, kind="ExternalOutput").ap()
        dbg_zp = nc.dram_tensor("dbg_zp", [128, n], FP32, kind="ExternalOutput").ap()
        dbg_zd = nc.dram_tensor("dbg_zd", [64, n], FP32, kind="ExternalOutput").ap()
        dbg_zback = nc.dram_tensor("dbg_zback", [64, n], FP32, kind="ExternalOutput").ap()
        dbg_zb = nc.dram_tensor("dbg_zb", [128, n], FP32, kind="ExternalOutput").ap()

    xT_r = xT.rearrange("(kc p) n -> kc p n", p=128)
    wqT_r = wqT.rearrange("(kc p) h -> kc p h", p=128)
    wkT_r = wkT.rearrange("(kc p) h -> kc p h", p=128)
    wvT_r = wvT.rearrange("(kc p) h -> kc p h", p=128)
    woutT_r = woutT.rearrange("(kc p) c2 -> kc p c2", p=128)

    with tile.TileContext(nc) as tc:
        persist = tc.alloc_tile_pool(name="persist", bufs=1)
        qT_sb = persist.tile([128, mt_q, n], BF16, name="qT_sb")
        kT_sb = persist.tile([128, mt_q, n], BF16, name="kT_sb")
        v_sb = persist.tile([128, n_jt, hd_l], BF16, name="v_sb")
        woutT_sb = persist.tile([128, kc_w, c], BF16, name="woutT_sb")
        atsc = [persist.tile([128, n], BF16, name=f"atsc{i}", tag=f"atsc{i}")
                for i in range(mt_q)]
        ones_sb = persist.tile([128, 1], BF16, name="ones_sb")
        nc.vector.memset(ones_sb[:, :], 1.0)

        ph1 = tc.alloc_tile_pool(name="ph1", bufs=1)
        xT_sb = ph1.tile([128, kc_x, n], BF16, name="xT_sb")
        wqT_sb = ph1.tile([128, kc_x, hd_l], BF16, name="wqT_sb")
        wkT_sb = ph1.tile([128, kc_x, hd_l], BF16, name="wkT_sb")
        wvT_sb = ph1.tile([128, kc_x, hd_l], BF16, name="wvT_sb")

        nh = n // 2
        for kc in range(kc_x):
            nc.sync.dma_start(out=wqT_sb[:, kc, :], in_=wqT_r[kc])
            nc.gpsimd.dma_start(out=wkT_sb[:, kc, :], in_=wkT_r[kc])
            eng = nc.sync if kc % 2 == 0 else nc.gpsimd
            eng.dma_start(out=xT_sb[:, kc, 0:nh], in_=xT_r[kc][:, 0:nh])
        for kc in range(kc_x):
            nc.sync.dma_start(out=xT_sb[:, kc, nh:n], in_=xT_r[kc][:, nh:n])
            nc.gpsimd.dma_start(out=wvT_sb[:, kc, :], in_=wvT_r[kc])
        for kc in range(kc_w):
            nc.gpsimd.dma_start(out=woutT_sb[:, kc, :], in_=woutT_r[kc])

        # ---- phases 1+2 interleaved: qkv chains feed the attention pipeline ----
        # Attention is ScalarE-bound (exp). qkv matmul chains are emitted inside
        # the attention loop so the PE computes projections while ScalarE exps.
        stp = tc.alloc_tile_pool(name="stp", bufs=2, space="PSUM")
        atp = tc.alloc_tile_pool(name="atp", bufs=1, space="PSUM")
        qkvp = tc.alloc_tile_pool(name="qkvp", bufs=2, space="PSUM")
        ptp = tc.alloc_tile_pool(name="ptp", bufs=12)
        zpp = tc.alloc_tile_pool(name="zpp", bufs=2)
        zdp = tc.alloc_tile_pool(name="zdp", bufs=2)
        zbp = tc.alloc_tile_pool(name="zbp", bufs=2)
        zdramp = tc.alloc_tile_pool(name="zdramp", bufs=2, space="DRAM")
        osbp = tc.alloc_tile_pool(name="osbp", bufs=3)

        def qk_chain(wsb, dst, mt, ip):
            """One q/k projection chain: 2 psum tiles, LDW amortized x2."""
            ps = [qkvp.tile([128, 512], FP32, name="ps", tag="qkv")
                  for _ in range(2)]
            for kc in range(kc_x):
                for i2 in range(2):
                    ich = ip * 2 + i2
                    nc.tensor.matmul(
                        ps[i2][:, :],
                        lhsT=wsb[:, kc, mt * 128:(mt + 1) * 128],
                        rhs=xT_sb[:, kc, ich * 512:(ich + 1) * 512],
                        start=(kc == 0), stop=(kc == kc_x - 1))
            for i2 in range(2):
                ich = ip * 2 + i2
                nc.vector.tensor_copy(
                    dst[:, mt, ich * 512:(ich + 1) * 512], ps[i2][:, :])

        def qk_chain1(wsb, dst, mt, ich):
            """Single-slot projection chain for startup priming."""
            ps1 = qkvp.tile([128, 512], FP32, name="ps1", tag="qkv")
            for kc in range(kc_x):
                nc.tensor.matmul(
                    ps1[:, :],
                    lhsT=wsb[:, kc, mt * 128:(mt + 1) * 128],
                    rhs=xT_sb[:, kc, ich * 512:(ich + 1) * 512],
                    start=(kc == 0), stop=(kc == kc_x - 1))
            nc.vector.tensor_copy(
                dst[:, mt, ich * 512:(ich + 1) * 512], ps1[:, :])

        def v_chain(nt):
            v_ps = qkvp.tile([128, hd_l], FP32, name="v_ps", tag="qkv")
            for kc in range(kc_x):
                nc.tensor.matmul(
                    v_ps[:, :],
                    lhsT=xT_sb[:, kc, nt * 128:(nt + 1) * 128],
                    rhs=wvT_sb[:, kc, :],
                    start=(kc == 0), stop=(kc == kc_x - 1))
            nc.vector.tensor_copy(v_sb[:, nt, :], v_ps[:, :])

        def op_chain(it, cch, pool=None, tag="qkv"):
            """One output-projection chain: accumulate over kc, copy out, DMA."""
            ops = (pool or qkvp).tile([128, 512], FP32, name="ops", tag=tag)
            for kc in range(kc_w):
                nc.tensor.matmul(
                    ops[:, :],
                    lhsT=atsc[kc][:, it * 128:(it + 1) * 128],
                    rhs=woutT_sb[:, kc, cch * 512:(cch + 1) * 512],
                    start=(kc == 0), stop=(kc == kc_w - 1))
            o_sb = osbp.tile([128, 512], FP32, name="o_sb", tag="osb")
            nc.vector.tensor_copy(o_sb[:, :], ops[:, :])
            nc.sync.dma_start(
                out=out[it * 128:(it + 1) * 128, cch * 512:(cch + 1) * 512],
                in_=o_sb[:, :])

        # work list of deferred qkv chains, emitted inside the attention loop
        work = []
        for ip in range(1, n_ich // 2):
            work.append(("k", 0, ip))
        for nt in range(2, n_jt):
            work.append(("v", nt))
        for ip in range(1, n_ich // 2):
            work.append(("q", 0, ip))
        for mt in range(1, mt_q):
            for ip in range(n_ich // 2):
                work.append(("q", mt, ip))
                work.append(("k", mt, ip))
        wi = 0

        def emit_work(k_items):
            nonlocal wi
            for _ in range(k_items):
                if wi >= len(work):
                    return
                item = work[wi]
                wi += 1
                if item[0] == "v":
                    v_chain(item[1])
                else:
                    wsb, dst = ((wqT_sb, qT_sb) if item[0] == "q"
                                else (wkT_sb, kT_sb))
                    qk_chain(wsb, dst, item[1], item[2])

        # prime: only what the first scores need, q and k concurrent via
        # single-slot chains (a 2-slot chain would serialize q behind k)
        qk_chain1(wqT_sb, qT_sb, 0, 0)
        qk_chain1(wkT_sb, kT_sb, 0, 0)
        qk_chain1(wqT_sb, qT_sb, 0, 1)
        qk_chain1(wkT_sb, kT_sb, 0, 1)
        v_chain(0)
        v_chain(1)

        zdance = zdp.tile([64, 1024], FP32, name="zdance", tag="zd")
        zdance_r = zdp.tile([64, 1024], FP32, name="zdance_r", tag="zdr")
        zback = zdp.tile([64, 1024], FP32, name="zback", tag="zbk")
        nc.vector.memset(zdance[:, :], 1.0)
        nc.vector.memset(zdance_r[:, :], 1.0)

        for pt in range(pairs):
            roA, roB = 0, 64
            hA, hB = 2 * pt, 2 * pt + 1
            zpA = zpp.tile([128, n], BF16, name="zpA", tag="zpA")
            zpB = zpp.tile([128, n], BF16, name="zpB", tag="zpB")
            for ih in range(n_ih):
                at = atp.tile([128, 1024], FP32, name="at", tag="at")
                pending = None  # values lag scores by one jt to keep PE fed
                for jt in range(n_jt):
                    stA = stp.tile([128, 1024], FP32, name="stA", tag="st")
                    stB = stp.tile([128, 1024], FP32, name="stB", tag="st")
                    for s2 in range(2):
                        ich = ih * 2 + s2
                        nc.tensor.matmul(
                            stA[:, s2 * 512:(s2 + 1) * 512],
                            lhsT=kT_sb[roA:roA + 64, pt, jt * 128:(jt + 1) * 128],
                            rhs=qT_sb[roA:roA + 64, pt, ich * 512:(ich + 1) * 512],
                            start=True, stop=True)
                        nc.tensor.matmul(
                            stB[:, s2 * 512:(s2 + 1) * 512],
                            lhsT=kT_sb[roB:roB + 64, pt, jt * 128:(jt + 1) * 128],
                            rhs=qT_sb[roB:roB + 64, pt, ich * 512:(ich + 1) * 512],
                            start=True, stop=True)
                    ptA = ptp.tile([128, 1024], BF16, name="ptA", tag="pt")
                    ptB = ptp.tile([128, 1024], BF16, name="ptB", tag="pt")
                    nc.scalar.activation(ptA[:, :], stA[:, :],
                                         mybir.ActivationFunctionType.Exp)
                    nc.scalar.activation(ptB[:, :], stB[:, :],
                                         mybir.ActivationFunctionType.Exp)
                    sl = slice(ih * 1024, (ih + 1) * 1024)
                    if jt == 0:
                        nc.vector.tensor_copy(zpA[:, sl], ptA[:, :])
                        nc.vector.tensor_copy(zpB[:, sl], ptB[:, :])
                    else:
                        nc.vector.tensor_add(zpA[:, sl], zpA[:, sl], ptA[:, :])
                        nc.vector.tensor_add(zpB[:, sl], zpB[:, sl], ptB[:, :])
                    def emit_values(vjt, vptA, vptB):
                        vA = v_sb[:, vjt, hA * 64:hA * 64 + 64]
                        vB = v_sb[:, vjt, hB * 64:hB * 64 + 64]
                        for s2 in range(2):
                            csl = slice(s2 * 512, (s2 + 1) * 512)
                            nc.tensor.matmul(
                                at[roA:roA + 64, csl], lhsT=vA,
                                rhs=vptA[:, csl],
                                start=(vjt == 0), stop=(vjt == n_jt - 1),
                                skip_group_check=True)
                            nc.tensor.matmul(
                                at[roB:roB + 64, csl], lhsT=vB,
                                rhs=vptB[:, csl],
                                start=(vjt == 0), stop=(vjt == n_jt - 1),
                                skip_group_check=True)
                    if pending is not None:
                        emit_values(*pending)
                    pending = (jt, ptA, ptB)
                    if wi < len(work) and (pt, ih) == (0, 0) \
                            and (work[wi][0] == "v" or work[wi][1] == 0):
                        emit_work(1)
                    elif jt % 4 == 3:
                        emit_work(1)
                    if (n_ih >= 2 and (pt, ih) == (pairs - 1, n_ih - 1)
                            and wi >= len(work)):
                        op_chain(jt // 2, jt % 2)
                if pending is not None:
                    emit_values(*pending)
                # Z for this i-half: column sums + wide reciprocal + broadcast
                for hh, zp in ((0, zpA), (1, zpB)):
                    for s2 in range(2):
                        ich = ih * 2 + s2
                        cs = qkvp.tile([1, 512], FP32, name="cs", tag="qkv")
                        nc.tensor.matmul(
                            cs[:, :],
                            lhsT=ones_sb[:, :],
                            rhs=zp[:, ich * 512:(ich + 1) * 512],
                            start=True, stop=True)
                        nc.vector.tensor_copy(
                            zdance[32 * hh:32 * hh + 1,
                                   s2 * 512:(s2 + 1) * 512],
                            cs[:, :])
                emit_work(2)
                nc.vector.transpose(zdance_r[:, :], zdance[:, :])
                rview = zdance_r.rearrange("p (b r) -> p b r", r=32)
                nc.vector.reciprocal(rview[:, :, 0:1], rview[:, :, 0:1])
                nc.vector.transpose(zback[:, :], zdance_r[:, :])
                zb = zbp.tile([128, 1024], FP32, name="zb", tag="zb")
                zd2 = zdramp.tile([2, 1024], FP32, name="zd2", tag="zd2")
                nc.sync.dma_start(out=zd2[:, :], in_=zback[0:64:32, :])
                for g in range(2):
                    row = zd2[g:g + 1, :]
                    bc = bass.AP(tensor=row.tensor, offset=row.offset,
                                 ap=[[0, 64], row.ap[1]])
                    nc.sync.dma_start(out=zb[g * 64:(g + 1) * 64, :], in_=bc)
                nc.vector.tensor_mul(atsc[pt][:, ih * 1024:(ih + 1) * 1024],
                                     at[:, :], zb[:, :])
        emit_work(len(work))
        done_its = set(range(8)) if n_ih >= 2 else set()
        alt = 0
        for it in range(n // 128):
            if it in done_its:
                continue
            for cch in range(n_cch):
                if alt % 2 == 0:
                    op_chain(it, cch)
                else:
                    op_chain(it, cch, pool=stp, tag="st")
                alt += 1
        osbp.release()
        zdramp.release()
        zbp.release()
        zdp.release()
        zpp.release()
        ptp.release()
        qkvp.release()
        atp.release()
        stp.release()
        ph1.release()
        persist.release()

    nc.compile()
    return nc


def make_in_maps(x, Wq, Wk, Wv, Wout, n=2048, c=1024, heads=16, d=64):
    """Shard + pre-transpose + cast inputs for the 8 cores."""
    s = float(d) ** -0.25
    hd_l = (heads // 2) * d
    wT = {}
    for g in range(2):
        sl = slice(g * hd_l, (g + 1) * hd_l)
        wT[g] = (
            np.ascontiguousarray((Wq[sl] * s).T).astype(NP_BF16),
            np.ascontiguousarray((Wk[sl] * s).T).astype(NP_BF16),
            np.ascontiguousarray(Wv[sl].T).astype(NP_BF16),
            np.ascontiguousarray(Wout.T[sl]).astype(NP_BF16),
        )
    in_maps = []
    for core in range(N_CORES):
        b = core // 2
        g = core % 2
        wq, wk, wv, wo = wT[g]
        in_maps.append({
            "xT": np.ascontiguousarray(x[b].T).astype(NP_BF16),
            "wqT": wq, "wkT": wk, "wvT": wv, "woutT": wo,
        })
    return in_maps


_NC_CACHE = {}


def kernel(x, Wq, Wk, Wv, Wout, bout):
    global LAST_EXEC_TIME_NS
    b, n, c = x.shape
    heads = 16
    d = 64
    hd_l = (heads // 2) * d

    if "nc" not in _NC_CACHE:
        _NC_CACHE["nc"] = build_nc(n=n, c=c, hd_l=hd_l)
    nc = _NC_CACHE["nc"]

    in_maps = make_in_maps(np.asarray(x, np.float32), np.asarray(Wq, np.float32),
                           np.asarray(Wk, np.float32), np.asarray(Wv, np.float32),
                           np.asarray(Wout, np.float32), n=n, c=c, heads=heads, d=d)

    profile = os.environ.get("BASS_KERNEL_PROFILE", "0") == "1"
    if profile:
        _install_ntff_shim()
    res = bass_utils.run_bass_kernel_spmd(
        nc, in_maps, core_ids=list(range(N_CORES)), trace=profile)
    LAST_EXEC_TIME_NS = res.exec_time_ns

    bout = np.asarray(bout, np.float32)
    out = np.empty((b, n, c), np.float32)
    for bb in range(b):
        out[bb] = res.results[2 * bb]["out"] + res.results[2 * bb + 1]["out"] + bout
    return out
